# revision 25
# baseline (speedup 1.0000x reference)
"""Additive attention (tanh-score) kernel for one TRN2 chip (8 NeuronCores).

scores[b,q,k] = sum_h w_v[h] * tanh(qp[b,q,h] + kp[b,k,h])
out = softmax_k(mask(scores)) @ values

Sharding: over the n_q axis (32 query rows per core); every core sees all 16
batches, so the per-batch valid_lens become compile-time constants shared by
all cores (SPMD-safe), and masked key columns (k >= valid_lens[b]) are simply
never computed -- exactly matching the reference, whose masked scores of -1e6
underflow to softmax weight 0.0 in fp32.

Structure per batch:
 - prologue: DMA loads, PE transposes of queries/keys, fp32 projections
 - hot loop (per h-chunk, super-group of 16 q):
     DVE path: 16x tensor_scalar broadcast-add (bf16) + one big ACT tanh
     ACT path (tuned fraction, balances DVE vs ACT): 16x ACT tanh with
       per-partition bias fusing the add
     PE: matvec per (q, k-chunk) with the tanh tile stationary and w_v
       moving -> scoresT columns in PSUM
 - epilogue: scoresT -> [32, L] via PE transpose, softmax (ACT exp with
   fused row-sum), attn transpose, fp32 attn @ values, scale by 1/rowsum

Batch b+1's prologue is emitted before batch b's hot loop so the in-order
engines overlap the next batch's PE prologue with this batch's DVE/ACT work.
"""

import os
import numpy as np

_NCORES = 8


def _register_ntff_hook():
    """Register the axon NTFF profiling hook if the image's antenv lacks it."""
    import sys, types

    try:
        from antenv.axon_hooks import get_axon_ntff_profile_hook  # noqa: F401
        return
    except ImportError:
        pass
    try:
        import trn_agent_boot.trn_boot as tb

        mod = types.ModuleType("antenv.axon_hooks")
        hook = tb._ntff_profile_via_ctypes("/opt/axon/libaxon_pjrt.so")
        mod.get_axon_ntff_profile_hook = lambda: hook
        mod.set_axon_ntff_profile_hook = lambda h: None
        sys.modules["antenv.axon_hooks"] = mod
    except Exception:
        pass


def _build_graph(B, NQL, NK, D, H, DV, lvals, lpads, act_frac=0.15):
    """Build the per-core Bass graph. lvals: exact per-batch valid lengths;
    lpads: padded extents (multiples of 8, in [8, NK]). Returns compiled nc."""
    import concourse.bass as bass
    import concourse.tile as tile
    from concourse import bacc, mybir, masks

    f32 = mybir.dt.float32
    bf16 = mybir.dt.bfloat16
    AF = mybir.ActivationFunctionType
    ALU = mybir.AluOpType
    AX = mybir.AxisListType
    PSUM = bass.MemorySpace.PSUM

    nc = bacc.Bacc(
        "TRN2", target_bir_lowering=False, debug=False, num_devices=_NCORES
    )

    q_d = nc.dram_tensor("queries", (B, NQL, D), f32, kind="ExternalInput")
    k_d = nc.dram_tensor("keys", (B, NK, D), f32, kind="ExternalInput")
    v_d = nc.dram_tensor("values", (B, NK, DV), f32, kind="ExternalInput")
    wq_d = nc.dram_tensor("W_q", (D, H), f32, kind="ExternalInput")
    wk_d = nc.dram_tensor("W_k", (D, H), f32, kind="ExternalInput")
    wv_d = nc.dram_tensor("w_v", (H,), f32, kind="ExternalInput")
    out_d = nc.dram_tensor("out", (B, NQL, DV), f32, kind="ExternalOutput")

    NDC = D // 128   # d chunks (contraction for projections)
    NHC = H // 128   # h chunks (partitions in main loop)
    QG = 16          # queries per tanh super-block

    n_blocks = B * (NQL // QG) * NHC
    n_act = int(round(act_frac * n_blocks))
    act_path = set()
    if n_act > 0:
        stride = n_blocks / n_act
        act_path = {int(i * stride) for i in range(n_act)}

    def geom(b):
        L = lvals[b]
        Lp = lpads[b]
        nkc = (L + 127) // 128
        kcs = [min(128, L - 128 * c) for c in range(nkc)]
        nkcp = (Lp + 127) // 128
        kcsp = [min(128, Lp - 128 * c) for c in range(nkcp)]
        return L, Lp, nkc, kcs, nkcp, kcsp

    with tile.TileContext(nc) as tc:
        with (
            tc.tile_pool(name="const", bufs=1) as constp,
            tc.tile_pool(name="stage", bufs=3) as stagep,
            tc.tile_pool(name="vpool", bufs=6) as vpool,
            tc.tile_pool(name="proj", bufs=2) as projp,
            tc.tile_pool(name="hot", bufs=6) as hotp,
            tc.tile_pool(name="soft", bufs=2) as softp,
            tc.tile_pool(name="pt", bufs=2, space=PSUM) as pt_ps,
            tc.tile_pool(name="pkp", bufs=1, space=PSUM) as pkp_ps,
            tc.tile_pool(name="psc", bufs=3, space=PSUM) as psc_ps,
            tc.tile_pool(name="pso", bufs=2, space=PSUM) as pso_ps,
        ):
            # ---- constants ----
            ident = constp.tile([128, 128], f32)
            masks.make_identity(nc, ident[:])

            wq_f = constp.tile([128, NDC, H], f32)
            nc.sync.dma_start(wq_f[:], wq_d.ap().rearrange("(c p) h -> p c h", p=128))
            wk_f = constp.tile([128, NDC, H], f32)
            nc.sync.dma_start(wk_f[:], wk_d.ap().rearrange("(c p) h -> p c h", p=128))
            wq_sb = constp.tile([128, NDC, H], bf16)
            nc.vector.tensor_copy(
                wq_sb[:].rearrange("p c h -> p (c h)"),
                wq_f[:].rearrange("p c h -> p (c h)"),
            )
            wk_sb = constp.tile([128, NDC, H], bf16)
            nc.vector.tensor_copy(
                wk_sb[:].rearrange("p c h -> p (c h)"),
                wk_f[:].rearrange("p c h -> p (c h)"),
            )
            ident_bf = constp.tile([128, 128], bf16)
            nc.vector.tensor_copy(ident_bf[:], ident[:])
            wv_f32 = constp.tile([128, NHC], f32)
            nc.sync.dma_start(wv_f32[:], wv_d.ap().rearrange("(c p) -> p c", p=128))
            wv_bf = constp.tile([128, NHC], bf16)
            nc.vector.tensor_copy(wv_bf[:], wv_f32[:])

            state = {}  # per-batch tiles from prologue

            def prologue(b):
                L, Lp, nkc, kcs, nkcp, kcsp = geom(b)

                qnat = stagep.tile([NQL, D], f32, tag="qnat")
                nc.sync.dma_start(qnat[:], q_d.ap()[b])
                qT = stagep.tile([128, NDC, NQL], bf16, tag="qT")
                for dc in range(NDC):
                    ps = pt_ps.tile([128, 128], f32, tag="tp")
                    nc.tensor.transpose(
                        ps[:, :NQL],
                        qnat[:, 128 * dc : 128 * (dc + 1)],
                        ident[:NQL, :NQL],
                    )
                    nc.vector.tensor_copy(qT[:, dc, :], ps[:, :NQL])

                kT = stagep.tile([128, NDC, Lp], bf16, tag="kT")
                for c in range(nkcp):
                    kc = kcsp[c]
                    knat = stagep.tile([128, D], f32, tag="knat")
                    nc.sync.dma_start(
                        knat[:kc, :], k_d.ap()[b, 128 * c : 128 * c + kc, :]
                    )
                    for dc in range(NDC):
                        ps = pt_ps.tile([128, 128], f32, tag="tp")
                        nc.tensor.transpose(
                            ps[:, :kc],
                            knat[:kc, 128 * dc : 128 * (dc + 1)],
                            ident[:kc, :kc],
                        )
                        nc.vector.tensor_copy(
                            kT[:, dc, 128 * c : 128 * c + kc], ps[:, :kc]
                        )

                vals = []
                for c in range(nkc):
                    kc = kcs[c]
                    vstg = stagep.tile([128, DV], f32, tag="vstg")
                    nc.sync.dma_start(
                        vstg[:kc, :], v_d.ap()[b, 128 * c : 128 * c + kc, :]
                    )
                    vbf = vpool.tile([128, DV], bf16, tag="vbf")
                    nc.gpsimd.tensor_copy(vbf[:kc, :], vstg[:kc, :])
                    vals.append(vbf)

                # fp32 projections; one start/stop per psum bank
                kp_ps = pkp_ps.tile([128, NHC, 256], f32, tag="kp")
                qp_ps = pt_ps.tile([128, 128], f32, tag="tp")
                for hc in range(NHC):
                    for dc in range(NDC):
                        nc.tensor.matmul(
                            kp_ps[:, hc, :Lp],
                            wk_sb[:, dc, 128 * hc : 128 * (hc + 1)],
                            kT[:, dc, :],
                            start=(hc == 0 and dc == 0),
                            stop=(hc == NHC - 1 and dc == NDC - 1),
                        )
                    for dc in range(NDC):
                        nc.tensor.matmul(
                            qp_ps[:, hc * NQL : (hc + 1) * NQL],
                            wq_sb[:, dc, 128 * hc : 128 * (hc + 1)],
                            qT[:, dc, :],
                            start=(hc == 0 and dc == 0),
                            stop=(hc == NHC - 1 and dc == NDC - 1),
                        )
                kp_bf = projp.tile([128, NHC, Lp], bf16, tag="kpbf")
                qp_f = projp.tile([128, NHC, NQL], f32, tag="qpf")
                for hc in range(NHC):
                    nc.scalar.copy(kp_bf[:, hc, :], kp_ps[:, hc, :Lp])
                nc.vector.tensor_copy(
                    qp_f[:].rearrange("p c q -> p (c q)"), qp_ps[:, : NHC * NQL]
                )
                state[b] = (vals, kp_bf, qp_f)

            def hot(b, blk0):
                L, Lp, nkc, kcs, nkcp, kcsp = geom(b)
                vals, kp_bf, qp_f = state[b]
                scT_ps = psc_ps.tile([128, nkcp, NQL], f32, tag="scT")
                blk = blk0
                for g in range(NQL // QG):
                    for hc in range(NHC):
                        f_t = hotp.tile([128, QG * Lp], bf16, tag="f")
                        if blk in act_path:
                            for j in range(QG):
                                q = g * QG + j
                                nc.scalar.activation(
                                    f_t[:, j * Lp : (j + 1) * Lp],
                                    kp_bf[:, hc, :],
                                    AF.Tanh,
                                    bias=qp_f[:, hc, q : q + 1],
                                )
                        else:
                            s_t = hotp.tile([128, QG * Lp], bf16, tag="s")
                            for j in range(QG):
                                q = g * QG + j
                                nc.vector.tensor_scalar(
                                    s_t[:, j * Lp : (j + 1) * Lp],
                                    kp_bf[:, hc, :],
                                    qp_f[:, hc, q : q + 1],
                                    None,
                                    ALU.add,
                                )
                            nc.scalar.activation(f_t[:], s_t[:], AF.Tanh)
                        blk += 1
                        for j in range(QG):
                            q = g * QG + j
                            for c in range(nkcp):
                                kc = kcsp[c]
                                nc.tensor.matmul(
                                    scT_ps[0:kc, c, q : q + 1],
                                    f_t[:, j * Lp + 128 * c : j * Lp + 128 * c + kc],
                                    wv_bf[:, hc : hc + 1],
                                    start=(g == 0 and hc == 0 and j == 0 and c == 0),
                                    stop=(
                                        g == NQL // QG - 1
                                        and hc == NHC - 1
                                        and j == QG - 1
                                        and c == nkcp - 1
                                    ),
                                )
                return scT_ps, blk

            def epilogue(b, scT_ps):
                L, Lp, nkc, kcs, nkcp, kcsp = geom(b)
                vals, kp_bf, qp_f = state.pop(b)

                # single whole-tile copy: depends on the stop matmul, so
                # the accumulation group is closed before any read. On
                # ScalarE (not DVE) so DVE's in-order stream never stalls
                # waiting for this batch's matvecs.
                scT_sb = softp.tile([128, nkcp, NQL], f32, tag="scTsb")
                nc.vector.tensor_copy(scT_sb[:], scT_ps[:])
                # scores at so_ps[:, :256], out accumulator at [:, 256:512]
                so_ps = pso_ps.tile([NQL, 256 + DV], f32, tag="so")
                for c in range(nkc):
                    kc = kcs[c]
                    nc.tensor.matmul(
                        so_ps[:, 128 * c : 128 * c + kc],
                        scT_sb[0:kc, c, :],
                        ident[0:kc, 0:kc],
                        is_transpose=True,
                        start=(c == 0),
                        stop=(c == nkc - 1),
                    )

                # no max-subtraction: |score| <= sum|w_v| ~= 13, exp
                # stays comfortably inside f32 range.
                p_t = softp.tile([NQL, L], bf16, tag="p")
                rsum = softp.tile([NQL, 1], f32, tag="rsum")
                nc.scalar.activation(
                    p_t[:], so_ps[:, :L], AF.Exp, accum_out=rsum[:]
                )
                rinv = softp.tile([NQL, 1], f32, tag="rinv")
                nc.vector.reciprocal(rinv[:], rsum[:])

                pT = softp.tile([128, nkc, NQL], bf16, tag="pT")
                for c in range(nkc):
                    kc = kcs[c]
                    ps = pt_ps.tile([128, 128], bf16, tag="tp")
                    nc.tensor.transpose(
                        ps[:kc, :NQL],
                        p_t[:, 128 * c : 128 * c + kc],
                        ident_bf[:NQL, :NQL],
                    )
                    nc.vector.tensor_copy(pT[:kc, c, :], ps[:kc, :NQL])
                for c in range(nkc):
                    kc = kcs[c]
                    nc.tensor.matmul(
                        so_ps[:, 256 : 256 + DV],
                        pT[:kc, c, :],
                        vals[c][:kc, :],
                        start=(c == 0),
                        stop=(c == nkc - 1),
                    )
                out_sb = softp.tile([NQL, DV], f32, tag="osb")
                nc.vector.tensor_scalar(
                    out_sb[:], so_ps[:, 256 : 256 + DV], rinv[:], None, ALU.mult
                )
                nc.sync.dma_start(out_d.ap()[b], out_sb[:])

            # software-pipelined emission: prologue(b+1) before hot(b);
            # epilogue(b) deferred until after hot(b+1) so in-order engines
            # reach its ops with the dependencies a full batch stale.
            prologue(0)
            blk = 0
            pending = None
            for b in range(B):
                if b + 1 < B:
                    prologue(b + 1)
                scT_ps, blk = hot(b, blk)
                if pending is not None:
                    epilogue(*pending)
                pending = (b, scT_ps)
            epilogue(*pending)

    nc.compile()
    return nc


_GRAPH_CACHE = {}


def _get_graph(key):
    if key not in _GRAPH_CACHE:
        B, NQL, NK, D, H, DV, lvals, lpads = key
        _GRAPH_CACHE[key] = _build_graph(
            B, NQL, NK, D, H, DV, list(lvals), list(lpads),
            act_frac=float(os.environ.get("KERNEL_ACT_FRAC", "0.15")),
        )
    return _GRAPH_CACHE[key]


def kernel(queries, keys, values, valid_lens, W_q, W_k, w_v):
    from concourse import bass_utils

    queries = np.ascontiguousarray(np.asarray(queries, dtype=np.float32))
    keys = np.ascontiguousarray(np.asarray(keys, dtype=np.float32))
    values = np.ascontiguousarray(np.asarray(values, dtype=np.float32))
    W_q = np.ascontiguousarray(np.asarray(W_q, dtype=np.float32))
    W_k = np.ascontiguousarray(np.asarray(W_k, dtype=np.float32))
    w_v = np.ascontiguousarray(np.asarray(w_v, dtype=np.float32))
    vl = np.asarray(valid_lens).astype(np.int64)

    B, NQ, D = queries.shape
    NK = keys.shape[1]
    DV = values.shape[2]
    H = W_q.shape[1]
    assert NQ % _NCORES == 0
    NQL = NQ // _NCORES

    lvals = tuple(int(min(NK, max(1, l))) for l in vl)
    lpads = tuple(int(min(NK, max(8, -(-l // 8) * 8))) for l in lvals)

    nc = _get_graph((B, NQL, NK, D, H, DV, lvals, lpads))

    in_maps = []
    for j in range(_NCORES):
        in_maps.append(
            {
                "queries": np.ascontiguousarray(
                    queries[:, j * NQL : (j + 1) * NQL, :]
                ),
                "keys": keys,
                "values": values,
                "W_q": W_q,
                "W_k": W_k,
                "w_v": w_v,
            }
        )

    trace = os.environ.get("BASS_KERNEL_TRACE") == "1"
    if trace:
        _register_ntff_hook()
    res = bass_utils.run_bass_kernel_spmd(
        nc, in_maps, core_ids=list(range(_NCORES)), trace=trace
    )
    kernel.last_results = res

    out = np.empty((B, NQ, DV), dtype=np.float32)
    for j in range(_NCORES):
        out[:, j * NQL : (j + 1) * NQL, :] = res.results[j]["out"]
    return out
